# revision 11
# baseline (speedup 1.0000x reference)
"""Trainium2 Bass kernel for nn_DrawImageLayer (draw Gaussian strokes, max over time).

Reference semantics:
  out[b,i,j,0] = min(1, max_t I[b,t] * exp(-g*(r_i - y[b,t])^2) * exp(-g*(r_j - x[b,t])^2))
  r_k = k/28 - 0.5, g = (28/2)^2 = 196, shapes B=1024, T=64, canvas 28x28.

Strategy: pure data parallel, 128 batch rows per NeuronCore (= SBUF
partitions) across 8 cores. Log domain so exp commutes with max:
  out = exp( max_t [ (lnI[t] - q_x[t,j]) - q_y[t,i] ] ),  q = g*(r-coord)^2
The min(.,1) clamp is dropped: I < 1 strictly => all log values < 0.

Cost model measured on this runtime (see microbench*.py): engine instructions
carry a large fixed cost (DVE TT ~30-80us, ACT ~40us) nearly independent of
element count, EXCEPT GPSIMD (Pool) ops (~2.8ns/elem, no fixed cost) and DMAs
(in ~2.5us; out ~7us unless WAW-serialized on the same DRAM range).
tensor_reduce cost scales with output-segment count (2x392 beats 1x784).
Blocking semaphore waits add ~25us; pre-satisfied waits ~3us. Engines overlap.

So: few large instructions; cheap prep on the Pool engine; single fused fp16
cube; two 392-segment reduces; everything double-buffered so consecutive reps
pipeline and all waits on the bottleneck engine (DVE) are pre-satisfied.

Per rep (per core):
  sync : in-dma xs[k%2]           (wait vcb>=k-1, presat)   ~2.5us
  pool : d12 = r' - coord'        (wait dsx, presat)        \
         q   = d12*d12  in-place                             ~40us total
         ex  = lnI - q_x          (inc gex)                 /
  dve  : cube[i,j,t] = ex - q_y   fp16 50176 (wait gex, presat)  ~30us
         img[k%2] = max_t cube    2 reduces, 392 segs each       ~78us
  act  : imgo[k%2] = Exp(img)     (wait vrd, blocking ~25us, off critical path)
         out-dma (program order, no wait)
Steady-state throughput = DVE cycle ~ 110-120us/rep.
"""

from contextlib import ExitStack

import numpy as np

import concourse.bass as bass
import concourse.mybir as mybir
from concourse.bass_utils import run_bass_kernel_spmd

SIZE = 28
T = 64
B = 1024
BC = 128  # batch rows per core
NCORES = 8
P2 = SIZE * SIZE
G = (SIZE / 2.0) ** 2
F32 = mybir.dt.float32
F16 = mybir.dt.float16
AO = mybir.AluOpType
AF = mybir.ActivationFunctionType

XCOLS = 3 * T + SIZE  # y(64) | x(64) | lnI(64) | r(28), all t innermost
D12 = 2 * T * SIZE  # 3584, layout (c, k, t)
EXN = SIZE * T  # 1792, layout (j, t)
CUBE = P2 * T  # 50176, layout (i, j, t), t innermost
IH = SIZE // 2  # image rows per reduce (14 -> 392 output segments)

_GRID = (np.arange(SIZE, dtype=np.float32) / SIZE - 0.5).astype(np.float32)


def _ap(t, offset, dims):
    """AP over an sbuf tensor: partition dim [row_pitch, 128] + free dims."""
    return bass.AP(t, offset, [[t.shape[1], BC]] + [list(d) for d in dims])


def build(rep: int = 1) -> bass.Bass:
    nc = bass.Bass()
    xin = nc.declare_dram_parameter("xin", [BC, XCOLS], F32, isOutput=False)
    out = nc.declare_dram_parameter("out", [BC, P2], F32, isOutput=True)

    with ExitStack() as ctx:
        # double-buffered small tensors (index k%2)
        xs = ctx.enter_context(nc.sbuf_tensor([BC, 2 * XCOLS], F32))
        d12 = ctx.enter_context(nc.sbuf_tensor([BC, 2 * D12], F32))
        ex = ctx.enter_context(nc.sbuf_tensor([BC, 2 * EXN], F32))
        cube = ctx.enter_context(nc.sbuf_tensor([BC, CUBE], F16))
        img = ctx.enter_context(nc.sbuf_tensor([BC, 2 * P2], F32))
        imgo = ctx.enter_context(nc.sbuf_tensor([BC, 2 * P2], F32))
        dsx = ctx.enter_context(nc.semaphore("dsx"))  # in-dma done
        gex = ctx.enter_context(nc.semaphore("gex"))  # pool ex done
        vcb = ctx.enter_context(nc.semaphore("vcb"))  # dve cube done
        vrd = ctx.enter_context(nc.semaphore("vrd"))  # dve reduces done
        aex = ctx.enter_context(nc.semaphore("aex"))  # act exp done
        dso = ctx.enter_context(nc.semaphore("dso"))  # out-dma done
        block = ctx.enter_context(nc.Block())

        @block.sync
        def _(sync):
            for k in range(rep):
                di = sync.dma_start(
                    out=_ap(xs, (k % 2) * XCOLS, [[1, XCOLS]]), in_=xin[:, :]
                )
                if k > 1:
                    # WAR with 2-rep slack: cube(k-2) done => pool(k-2) done
                    # => xs[k%2], d12[k%2], ex[k%2] all consumed
                    di._wait_ge(vcb, k - 1)
                di.then_inc(dsx, 16)
            sync.wait_ge(dsx, rep * 16)
            sync.wait_ge(dso, rep * 16)

        @block.gpsimd
        def _(gpsimd):
            for k in range(rep):
                o = (k % 2) * XCOLS
                od = (k % 2) * D12
                # d12[(c,kk,t)] = sqrt(g)*(r_kk - coord_c[t])  (host pre-scales)
                nc.gpsimd.tensor_tensor(
                    _ap(d12, od, [[1, D12]]),
                    _ap(xs, o + 3 * T, [[0, 2], [1, SIZE], [0, T]]),
                    _ap(xs, o, [[T, 2], [0, SIZE], [1, T]]),
                    AO.subtract,
                )._wait_ge(dsx, k * 16 + 16)
                # q = d12 * d12 = g*(r-coord)^2, in place
                nc.gpsimd.tensor_tensor(
                    _ap(d12, od, [[1, D12]]),
                    _ap(d12, od, [[1, D12]]),
                    _ap(d12, od, [[1, D12]]),
                    AO.mult,
                )
                # ex[(j,t)] = lnI[t] - q_x[(j,t)]
                nc.gpsimd.tensor_tensor(
                    _ap(ex, (k % 2) * EXN, [[1, EXN]]),
                    _ap(xs, o + 2 * T, [[0, SIZE], [1, T]]),
                    _ap(d12, od + T * SIZE, [[T, SIZE], [1, T]]),
                    AO.subtract,
                ).then_inc(gex, 1)

        @block.vector
        def _(vector):
            for k in range(rep):
                od = (k % 2) * D12
                # cube[(i,j,t)] = ex[(j,t)] - q_y[(i,t)]
                nc.vector.tensor_tensor(
                    _ap(cube, 0, [[1, CUBE]]),
                    _ap(ex, (k % 2) * EXN, [[0, SIZE], [T, SIZE], [1, T]]),
                    _ap(d12, od, [[T, SIZE], [0, SIZE], [1, T]]),
                    AO.subtract,
                )._wait_ge(gex, k + 1).then_inc(vcb, 1)
                for h in range(2):
                    red = nc.vector.tensor_reduce(
                        _ap(img, (k % 2) * P2 + h * IH * SIZE, [[1, IH * SIZE]]),
                        _ap(cube, h * IH * SIZE * T, [[SIZE * T, IH], [T, SIZE], [1, T]]),
                        mybir.AxisListType.X,
                        AO.max,
                    )
                    if h == 0 and k > 1:
                        # WAR (2-rep slack): Exp(k-2) must have read img[k%2]
                        red._wait_ge(aex, k - 1)
                    if h == 1 and k > 1:
                        # WAR (2-rep slack): out-dma(k-2) must have read
                        # imgo[k%2]; red_h1 -> vrd -> Exp(k) orders it
                        red._wait_ge(dso, (k - 1) * 16)
                red.then_inc(vrd, 1)

        @block.scalar
        def _(scalar):
            for k in range(rep):
                o = (k % 2) * P2
                nc.scalar.activation(
                    _ap(imgo, o, [[1, P2]]),
                    _ap(img, o, [[1, P2]]),
                    AF.Exp,
                )._wait_ge(vrd, k + 1).then_inc(aex, 1)
                # out-dma from the ACT queue: program order after Exp, no wait
                nc.scalar.dma_start(
                    out=out[:, :], in_=_ap(imgo, o, [[1, P2]])
                ).then_inc(dso, 16)

    return nc


def make_in_maps(x: np.ndarray) -> list:
    """Shard x (1024, 64, 3) -> per-core host-prepped maps.

    Per core [128, 220] fp32: sqrt(g)*y[t] | sqrt(g)*x[t] | ln(I[t]) |
    sqrt(g)*grid, t innermost.
    """
    x = np.asarray(x, dtype=np.float32)
    maps = []
    sg = np.float32(np.sqrt(G))
    with np.errstate(divide="ignore"):
        lnI = np.log(x[:, :, 2]).astype(np.float32)  # (B, T); -inf ok
    for c in range(NCORES):
        sl = slice(c * BC, (c + 1) * BC)
        xc = np.empty((BC, XCOLS), np.float32)
        xc[:, 0:T] = sg * x[sl, :, 1]  # sqrt(g)*y
        xc[:, T : 2 * T] = sg * x[sl, :, 0]  # sqrt(g)*x
        xc[:, 2 * T : 3 * T] = lnI[sl]
        xc[:, 3 * T :] = sg * _GRID[None, :]
        maps.append({"xin": np.ascontiguousarray(xc)})
    return maps


def kernel(x: np.ndarray) -> np.ndarray:
    """Full inputs in, full output out: (1024, 64, 3) f32 -> (1024, 28, 28, 1) f32."""
    x = np.asarray(x, dtype=np.float32)
    assert x.shape == (B, T, 3), x.shape
    nc = build(rep=1)
    res = run_bass_kernel_spmd(nc, make_in_maps(x), list(range(NCORES)))
    outs = [res.results[c]["out"].reshape(BC, SIZE, SIZE, 1) for c in range(NCORES)]
    return np.concatenate(outs, axis=0)


# revision 13
# speedup vs baseline: 1.4504x; 1.4504x over previous
"""Trainium2 Bass kernel for nn_DrawImageLayer (draw Gaussian strokes, max over time).

Reference semantics:
  out[b,i,j,0] = min(1, max_t I[b,t] * exp(-g*(r_i - y[b,t])^2) * exp(-g*(r_j - x[b,t])^2))
  r_k = k/28 - 0.5, g = (28/2)^2 = 196, shapes B=1024, T=64, canvas 28x28.

Strategy: pure data parallel, 128 batch rows per NeuronCore (= SBUF
partitions) across 8 cores. Log domain so exp commutes with max:
  out = exp( max_t [ (lnI[t] - q_x[t,j]) - q_y[t,i] ] ),  q = g*(r-coord)^2
The min(.,1) clamp is dropped: I < 1 strictly => all log values < 0.

Cost model measured on this runtime (see microbench*.py): engine instructions
carry a large fixed cost (DVE TT ~30-80us, ACT ~40us) nearly independent of
element count, EXCEPT GPSIMD (Pool) ops (~2.8ns/elem, no fixed cost) and DMAs
(in ~2.5us; out ~7us unless WAW-serialized on the same DRAM range).
tensor_reduce cost scales with output-segment count (2x392 beats 1x784).
Blocking semaphore waits add ~25us; pre-satisfied waits ~3us. Engines overlap.

So: few large instructions; cheap prep on the Pool engine; single fused fp16
cube; two 392-segment reduces; everything double-buffered so consecutive reps
pipeline and all waits on the bottleneck engine (DVE) are pre-satisfied.

Per rep (per core):
  sync : in-dma xs[k%2]           (wait vcb>=k-1, presat)   ~2.5us
  pool : d12 = r' - coord'        (wait dsx, presat)        \
         q   = d12*d12  in-place                             ~40us total
         ex  = lnI - q_x          (inc gex)                 /
  dve  : cube[i,j,t] = ex - q_y   fp16 50176 (wait gex, presat)  ~30us
         img[k%2] = max_t cube    2 reduces, 392 segs each       ~78us
  act  : imgo[k%2] = Exp(img)     (wait vrd, blocking ~25us, off critical path)
         out-dma (program order, no wait)
Steady-state throughput = DVE cycle ~ 110-120us/rep.
"""

from contextlib import ExitStack

import numpy as np

import concourse.bass as bass
import concourse.mybir as mybir
from concourse.bass_utils import run_bass_kernel_spmd

SIZE = 28
T = 64
B = 1024
BC = 128  # batch rows per core
NCORES = 8
P2 = SIZE * SIZE
G = (SIZE / 2.0) ** 2
F32 = mybir.dt.float32
F16 = mybir.dt.float16
AO = mybir.AluOpType
AF = mybir.ActivationFunctionType

XCOLS = 3 * T + SIZE  # y(64) | x(64) | lnI(64) | r(28), all t innermost
D12 = 2 * T * SIZE  # 3584, layout (c, k, t)
EXN = SIZE * T  # 1792, layout (j, t)
CUBE = P2 * T  # 50176, layout (i, j, t), t innermost
IH = SIZE // 2  # image rows per reduce (14 -> 392 output segments)

_GRID = (np.arange(SIZE, dtype=np.float32) / SIZE - 0.5).astype(np.float32)


def _ap(t, offset, dims):
    """AP over an sbuf tensor: partition dim [row_pitch, 128] + free dims."""
    return bass.AP(t, offset, [[t.shape[1], BC]] + [list(d) for d in dims])


def build(rep: int = 1) -> bass.Bass:
    nc = bass.Bass()
    xin = nc.declare_dram_parameter("xin", [BC, XCOLS], F32, isOutput=False)
    out = nc.declare_dram_parameter("out", [BC, P2], F32, isOutput=True)

    with ExitStack() as ctx:
        # double-buffered small tensors (index k%2)
        xs = ctx.enter_context(nc.sbuf_tensor([BC, 2 * XCOLS], F32))
        d12 = ctx.enter_context(nc.sbuf_tensor([BC, 2 * D12], F32))
        ex = ctx.enter_context(nc.sbuf_tensor([BC, 2 * EXN], F32))
        cube = ctx.enter_context(nc.sbuf_tensor([BC, CUBE], F16))
        img = ctx.enter_context(nc.sbuf_tensor([BC, 2 * P2], F32))
        imgo = ctx.enter_context(nc.sbuf_tensor([BC, 2 * P2], F32))
        dsx = ctx.enter_context(nc.semaphore("dsx"))  # in-dma done
        gex = ctx.enter_context(nc.semaphore("gex"))  # pool ex done
        vcb = ctx.enter_context(nc.semaphore("vcb"))  # dve cube done
        vrd = ctx.enter_context(nc.semaphore("vrd"))  # dve reduces done
        aex = ctx.enter_context(nc.semaphore("aex"))  # act exp done
        dso = ctx.enter_context(nc.semaphore("dso"))  # out-dma done
        block = ctx.enter_context(nc.Block())

        def in_dma(sync, k):
            di = sync.dma_start(
                out=_ap(xs, (k % 2) * XCOLS, [[1, XCOLS]]), in_=xin[:, :]
            )
            if k > 1:
                # WAR with 2-rep slack: cube(k-2) done => pool(k-2) done
                # => xs[k%2], d12[k%2], ex[k%2] all consumed
                di._wait_ge(vcb, k - 1)
            di.then_inc(dsx, 16)

        @block.sync
        def _(sync):
            # prefetch 2 in-dmas so the out-dma's blocking wait never starves
            # the pool engine of input
            for k in range(min(rep, 2)):
                in_dma(sync, k)
            for k in range(rep):
                sync.dma_start(
                    out=out[:, :], in_=_ap(imgo, (k % 2) * P2, [[1, P2]])
                )._wait_ge(aex, k + 1).then_inc(dso, 16)
                if k + 2 < rep:
                    in_dma(sync, k + 2)
            sync.wait_ge(dsx, rep * 16)
            sync.wait_ge(dso, rep * 16)

        @block.gpsimd
        def _(gpsimd):
            for k in range(rep):
                o = (k % 2) * XCOLS
                od = (k % 2) * D12
                # d12[(c,kk,t)] = sqrt(g)*(r_kk - coord_c[t])  (host pre-scales)
                nc.gpsimd.tensor_tensor(
                    _ap(d12, od, [[1, D12]]),
                    _ap(xs, o + 3 * T, [[0, 2], [1, SIZE], [0, T]]),
                    _ap(xs, o, [[T, 2], [0, SIZE], [1, T]]),
                    AO.subtract,
                )._wait_ge(dsx, k * 16 + 16)
                # q = d12 * d12 = g*(r-coord)^2, in place
                nc.gpsimd.tensor_tensor(
                    _ap(d12, od, [[1, D12]]),
                    _ap(d12, od, [[1, D12]]),
                    _ap(d12, od, [[1, D12]]),
                    AO.mult,
                )
                # ex[(j,t)] = lnI[t] - q_x[(j,t)]
                nc.gpsimd.tensor_tensor(
                    _ap(ex, (k % 2) * EXN, [[1, EXN]]),
                    _ap(xs, o + 2 * T, [[0, SIZE], [1, T]]),
                    _ap(d12, od + T * SIZE, [[T, SIZE], [1, T]]),
                    AO.subtract,
                ).then_inc(gex, 1)

        @block.vector
        def _(vector):
            for k in range(rep):
                od = (k % 2) * D12
                # cube[(i,j,t)] = ex[(j,t)] - q_y[(i,t)]
                nc.vector.tensor_tensor(
                    _ap(cube, 0, [[1, CUBE]]),
                    _ap(ex, (k % 2) * EXN, [[0, SIZE], [T, SIZE], [1, T]]),
                    _ap(d12, od, [[T, SIZE], [0, SIZE], [1, T]]),
                    AO.subtract,
                )._wait_ge(gex, k + 1).then_inc(vcb, 1)
                for h in range(2):
                    red = nc.vector.tensor_reduce(
                        _ap(img, (k % 2) * P2 + h * IH * SIZE, [[1, IH * SIZE]]),
                        _ap(cube, h * IH * SIZE * T, [[SIZE * T, IH], [T, SIZE], [1, T]]),
                        mybir.AxisListType.X,
                        AO.max,
                    )
                    if h == 0 and k > 1:
                        # WAR (2-rep slack): Exp(k-2) must have read img[k%2]
                        red._wait_ge(aex, k - 1)
                    if h == 1 and k > 1:
                        # WAR (2-rep slack): out-dma(k-2) must have read
                        # imgo[k%2]; red_h1 -> vrd -> Exp(k) orders it
                        red._wait_ge(dso, (k - 1) * 16)
                red.then_inc(vrd, 1)

        @block.scalar
        def _(scalar):
            for k in range(rep):
                o = (k % 2) * P2
                nc.scalar.activation(
                    _ap(imgo, o, [[1, P2]]),
                    _ap(img, o, [[1, P2]]),
                    AF.Exp,
                )._wait_ge(vrd, k + 1).then_inc(aex, 1)

    return nc


def make_in_maps(x: np.ndarray) -> list:
    """Shard x (1024, 64, 3) -> per-core host-prepped maps.

    Per core [128, 220] fp32: sqrt(g)*y[t] | sqrt(g)*x[t] | ln(I[t]) |
    sqrt(g)*grid, t innermost.
    """
    x = np.asarray(x, dtype=np.float32)
    maps = []
    sg = np.float32(np.sqrt(G))
    with np.errstate(divide="ignore"):
        lnI = np.log(x[:, :, 2]).astype(np.float32)  # (B, T); -inf ok
    for c in range(NCORES):
        sl = slice(c * BC, (c + 1) * BC)
        xc = np.empty((BC, XCOLS), np.float32)
        xc[:, 0:T] = sg * x[sl, :, 1]  # sqrt(g)*y
        xc[:, T : 2 * T] = sg * x[sl, :, 0]  # sqrt(g)*x
        xc[:, 2 * T : 3 * T] = lnI[sl]
        xc[:, 3 * T :] = sg * _GRID[None, :]
        maps.append({"xin": np.ascontiguousarray(xc)})
    return maps


def kernel(x: np.ndarray) -> np.ndarray:
    """Full inputs in, full output out: (1024, 64, 3) f32 -> (1024, 28, 28, 1) f32."""
    x = np.asarray(x, dtype=np.float32)
    assert x.shape == (B, T, 3), x.shape
    nc = build(rep=1)
    res = run_bass_kernel_spmd(nc, make_in_maps(x), list(range(NCORES)))
    outs = [res.results[c]["out"].reshape(BC, SIZE, SIZE, 1) for c in range(NCORES)]
    return np.concatenate(outs, axis=0)


# revision 15
# speedup vs baseline: 1.6443x; 1.1337x over previous
"""Trainium2 Bass kernel for nn_DrawImageLayer (draw Gaussian strokes, max over time).

Reference semantics:
  out[b,i,j,0] = min(1, max_t I[b,t] * exp(-g*(r_i - y[b,t])^2) * exp(-g*(r_j - x[b,t])^2))
  r_k = k/28 - 0.5, g = (28/2)^2 = 196, shapes B=1024, T=64, canvas 28x28.

Strategy: pure data parallel, 128 batch rows per NeuronCore (= SBUF
partitions) across 8 cores. Log domain so exp commutes with max:
  out = exp( max_t [ (lnI[t] - q_x[t,j]) - q_y[t,i] ] ),  q = g*(r-coord)^2
The min(.,1) clamp is dropped: I < 1 strictly => all log values < 0.

Cost model measured on this runtime (see microbench*.py): engine instructions
carry a large fixed cost (DVE TT ~30-80us, ACT ~40us) nearly independent of
element count, EXCEPT GPSIMD (Pool) ops (~2.8ns/elem, no fixed cost) and DMAs
(in ~2.5us; out ~7us unless WAW-serialized on the same DRAM range).
tensor_reduce cost scales with output-segment count (2x392 beats 1x784).
Blocking semaphore waits add ~25us; pre-satisfied waits ~3us. Engines overlap.

So: few large instructions; cheap prep on the Pool engine; single fused fp16
cube; two 392-segment reduces; everything double-buffered so consecutive reps
pipeline and all waits on the bottleneck engine (DVE) are pre-satisfied.

Per rep (per core):
  sync : in-dma xs[k%2]           (wait vcb>=k-1, presat)   ~2.5us
  pool : d12 = r' - coord'        (wait dsx, presat)        \
         q   = d12*d12  in-place                             ~40us total
         ex  = lnI - q_x          (inc gex)                 /
  dve  : cube[i,j,t] = ex - q_y   fp16 50176 (wait gex, presat)  ~30us
         img[k%2] = max_t cube    2 reduces, 392 segs each       ~78us
  act  : imgo[k%2] = Exp(img)     (wait vrd, blocking ~25us, off critical path)
         out-dma (program order, no wait)
Steady-state throughput = DVE cycle ~ 110-120us/rep.
"""

from contextlib import ExitStack

import numpy as np

import concourse.bass as bass
import concourse.mybir as mybir
from concourse.bass_utils import run_bass_kernel_spmd

SIZE = 28
T = 64
B = 1024
BC = 128  # batch rows per core
NCORES = 8
P2 = SIZE * SIZE
G = (SIZE / 2.0) ** 2
F32 = mybir.dt.float32
F16 = mybir.dt.float16
AO = mybir.AluOpType
AF = mybir.ActivationFunctionType

XCOLS = 3 * T + SIZE  # y(64) | x(64) | lnI(64) | r(28), all t innermost
D12 = 2 * T * SIZE  # 3584, layout (c, k, t)
EXN = SIZE * T  # 1792, layout (j, t)
CUBE = P2 * T  # 50176, layout (i, j, t), t innermost
IH = SIZE // 2  # image rows per reduce (14 -> 392 output segments)

_GRID = (np.arange(SIZE, dtype=np.float32) / SIZE - 0.5).astype(np.float32)


def _ap(t, offset, dims):
    """AP over an sbuf tensor: partition dim [row_pitch, 128] + free dims."""
    return bass.AP(t, offset, [[t.shape[1], BC]] + [list(d) for d in dims])


def build(rep: int = 1) -> bass.Bass:
    nc = bass.Bass()
    xin = nc.declare_dram_parameter("xin", [BC, XCOLS], F32, isOutput=False)
    out = nc.declare_dram_parameter("out", [BC, P2], F32, isOutput=True)

    with ExitStack() as ctx:
        # double-buffered small tensors (index k%2)
        xs = ctx.enter_context(nc.sbuf_tensor([BC, 2 * XCOLS], F32))
        d12 = ctx.enter_context(nc.sbuf_tensor([BC, 2 * D12], F32))
        ex = ctx.enter_context(nc.sbuf_tensor([BC, 2 * EXN], F32))
        cube = ctx.enter_context(nc.sbuf_tensor([BC, CUBE], F16))
        img = ctx.enter_context(nc.sbuf_tensor([BC, 2 * P2], F32))
        imgo = ctx.enter_context(nc.sbuf_tensor([BC, 2 * P2], F32))
        dsx = ctx.enter_context(nc.semaphore("dsx"))  # in-dma done
        gex = ctx.enter_context(nc.semaphore("gex"))  # pool ex done
        vcb = ctx.enter_context(nc.semaphore("vcb"))  # dve cube done
        vrd = ctx.enter_context(nc.semaphore("vrd"))  # dve reduces done
        aex = ctx.enter_context(nc.semaphore("aex"))  # act exp done
        dso = ctx.enter_context(nc.semaphore("dso"))  # out-dma done
        block = ctx.enter_context(nc.Block())

        def in_dma(sync, k):
            di = sync.dma_start(
                out=_ap(xs, (k % 2) * XCOLS, [[1, XCOLS]]), in_=xin[:, :]
            )
            if k > 1:
                # WAR with 2-rep slack: Exp(k-2) done => red/cube/pool(k-2)
                # done => xs[k%2], d12[k%2], ex[k%2], img[k%2] all consumed
                di._wait_ge(aex, k - 1)
            di.then_inc(dsx, 16)

        @block.sync
        def _(sync):
            # issue in-dma(k+2) BEFORE out-dma(k): the out-dma's blocking wait
            # must not starve the pool engine of the next rep's input
            for k in range(min(rep, 2)):
                in_dma(sync, k)
            for k in range(rep):
                if k + 2 < rep:
                    in_dma(sync, k + 2)
                sync.dma_start(
                    out=out[:, :], in_=_ap(imgo, (k % 2) * P2, [[1, P2]])
                )._wait_ge(aex, k + 1).then_inc(dso, 16)
            sync.wait_ge(dsx, rep * 16)
            sync.wait_ge(dso, rep * 16)

        @block.gpsimd
        def _(gpsimd):
            for k in range(rep):
                o = (k % 2) * XCOLS
                od = (k % 2) * D12
                # d12[(c,kk,t)] = sqrt(g)*(r_kk - coord_c[t])  (host pre-scales)
                nc.gpsimd.tensor_tensor(
                    _ap(d12, od, [[1, D12]]),
                    _ap(xs, o + 3 * T, [[0, 2], [1, SIZE], [0, T]]),
                    _ap(xs, o, [[T, 2], [0, SIZE], [1, T]]),
                    AO.subtract,
                )._wait_ge(dsx, k * 16 + 16)
                # q = d12 * d12 = g*(r-coord)^2, in place
                nc.gpsimd.tensor_tensor(
                    _ap(d12, od, [[1, D12]]),
                    _ap(d12, od, [[1, D12]]),
                    _ap(d12, od, [[1, D12]]),
                    AO.mult,
                )
                # ex[(j,t)] = lnI[t] - q_x[(j,t)]
                nc.gpsimd.tensor_tensor(
                    _ap(ex, (k % 2) * EXN, [[1, EXN]]),
                    _ap(xs, o + 2 * T, [[0, SIZE], [1, T]]),
                    _ap(d12, od + T * SIZE, [[T, SIZE], [1, T]]),
                    AO.subtract,
                ).then_inc(gex, 1)

        @block.vector
        def _(vector):
            for k in range(rep):
                od = (k % 2) * D12
                # cube[(i,j,t)] = ex[(j,t)] - q_y[(i,t)]
                nc.vector.tensor_tensor(
                    _ap(cube, 0, [[1, CUBE]]),
                    _ap(ex, (k % 2) * EXN, [[0, SIZE], [T, SIZE], [1, T]]),
                    _ap(d12, od, [[T, SIZE], [0, SIZE], [1, T]]),
                    AO.subtract,
                )._wait_ge(gex, k + 1).then_inc(vcb, 1)
                # single 784-segment reduce: beats 2x392 once the extra
                # dependent-instruction stall is accounted for
                red = nc.vector.tensor_reduce(
                    _ap(img, (k % 2) * P2, [[1, P2]]),
                    _ap(cube, 0, [[SIZE * T, SIZE], [T, SIZE], [1, T]]),
                    mybir.AxisListType.X,
                    AO.max,
                )
                if k > 1:
                    # WAR (2-rep slack): out-dma(k-2) must have read
                    # imgo[k%2]; red -> vrd -> Exp(k) orders it
                    red._wait_ge(dso, (k - 1) * 16)
                red.then_inc(vrd, 1)

        @block.scalar
        def _(scalar):
            for k in range(rep):
                o = (k % 2) * P2
                nc.scalar.activation(
                    _ap(imgo, o, [[1, P2]]),
                    _ap(img, o, [[1, P2]]),
                    AF.Exp,
                )._wait_ge(vrd, k + 1).then_inc(aex, 1)

    return nc


def make_in_maps(x: np.ndarray) -> list:
    """Shard x (1024, 64, 3) -> per-core host-prepped maps.

    Per core [128, 220] fp32: sqrt(g)*y[t] | sqrt(g)*x[t] | ln(I[t]) |
    sqrt(g)*grid, t innermost.
    """
    x = np.asarray(x, dtype=np.float32)
    maps = []
    sg = np.float32(np.sqrt(G))
    with np.errstate(divide="ignore"):
        lnI = np.log(x[:, :, 2]).astype(np.float32)  # (B, T); -inf ok
    for c in range(NCORES):
        sl = slice(c * BC, (c + 1) * BC)
        xc = np.empty((BC, XCOLS), np.float32)
        xc[:, 0:T] = sg * x[sl, :, 1]  # sqrt(g)*y
        xc[:, T : 2 * T] = sg * x[sl, :, 0]  # sqrt(g)*x
        xc[:, 2 * T : 3 * T] = lnI[sl]
        xc[:, 3 * T :] = sg * _GRID[None, :]
        maps.append({"xin": np.ascontiguousarray(xc)})
    return maps


def kernel(x: np.ndarray) -> np.ndarray:
    """Full inputs in, full output out: (1024, 64, 3) f32 -> (1024, 28, 28, 1) f32."""
    x = np.asarray(x, dtype=np.float32)
    assert x.shape == (B, T, 3), x.shape
    nc = build(rep=1)
    res = run_bass_kernel_spmd(nc, make_in_maps(x), list(range(NCORES)))
    outs = [res.results[c]["out"].reshape(BC, SIZE, SIZE, 1) for c in range(NCORES)]
    return np.concatenate(outs, axis=0)
